# revision 14
# baseline (speedup 1.0000x reference)
"""Trainium2 Bass kernel for nn_CMIA_2843268350555 (dual-branch spatial/freq attention).

Strategy: data-parallel over batch (16 samples / 8 cores = 2 per core).
All matmuls in float32r (11-bit mantissa, full PE rate at free-dim>=256).

Per-sample math (C=256 channels, HW=1024):
  vT_b    = (x_b.T @ w_bv.T)            [hw, c]   (b in {spa, frq})
  x       = w_cdc @ [x_spa; x_frq]      [c, hw]   (+b_cdc: no-op through LN)
  xn      = layernorm_rows(x)           [c, hw]   (affine folded into wqkTg)
  xnT     = transpose(xn)               [hw, c]
  qk      = (xnT.T @ wqkTg) = xn@..     [c, 2hw]  -> q [c,hw], k [c,hw]
  kT      = transpose(k)                [hw, c]
  kw_b    = (kT.T @ (scale*w_b.T))      [c, hw]   (associativity: avoids big logits mm)
  logits  = q.T @ kw_b                  [hw(n), hw(j)]
  att_b   = softmax_j(logits + b_b)
  out_b   = x_b + (vT_b.T @ att_b)      [c, hw]
"""
import numpy as np

import concourse.bacc as bacc
import concourse.mybir as mybir
import concourse.tile as tile
from concourse import bass_utils
from concourse.bass import ts, ds
from concourse.masks import make_identity

f32 = mybir.dt.float32
f32r = mybir.dt.float32r

B, C, H, W = 16, 256, 32, 32
HW = H * W           # 1024
J2 = 2 * HW          # 2048
NCORES = 8
BPC = B // NCORES    # samples per core
CC = C // 128        # 2 channel chunks
NCH = HW // 128      # 8 hw chunks
EPS = 1e-5


def _round_f32r(x: np.ndarray) -> np.ndarray:
    """RNE-round fp32 to fp32r (11 mantissa bits; low 12 bits zero)."""
    x = np.ascontiguousarray(x, dtype=np.float32)
    u = x.view(np.uint32)
    lsb = (u >> np.uint32(12)) & np.uint32(1)
    r = u + np.uint32(0x7FF) + lsb
    return (r & ~np.uint32(0xFFF)).view(np.float32)


_CACHE: dict = {}


def _build(flags, reps=1):
    has_qkb, has_bspa, has_bfrq, has_bsv, has_bfv = flags
    any_bias = any(flags)

    nc = bacc.Bacc("TRN2", target_bir_lowering=False, debug=False,
                   enable_asserts=True, num_devices=NCORES)
    xs_d = nc.dram_tensor("xs", [BPC, C, HW], f32r, kind="ExternalInput").ap()
    xf_d = nc.dram_tensor("xf", [BPC, C, HW], f32r, kind="ExternalInput").ap()
    wcdc_d = nc.dram_tensor("wcdcT", [2 * C, C], f32r, kind="ExternalInput").ap()
    wsv_d = nc.dram_tensor("wsvT", [C, C], f32r, kind="ExternalInput").ap()
    wfv_d = nc.dram_tensor("wfvT", [C, C], f32r, kind="ExternalInput").ap()
    wqk_d = nc.dram_tensor("wqkTg", [HW, J2], f32r, kind="ExternalInput").ap()
    wspa_d = nc.dram_tensor("wspaT", [HW, HW], f32r, kind="ExternalInput").ap()
    wfrq_d = nc.dram_tensor("wfrqT", [HW, HW], f32r, kind="ExternalInput").ap()
    qkb_d = bspa_d = bfrq_d = bsv_d = bfv_d = None
    if has_qkb:
        qkb_d = nc.dram_tensor("qkb", [1, J2], f32r, kind="ExternalInput").ap()
    if has_bspa:
        bspa_d = nc.dram_tensor("bspa", [1, HW], f32r, kind="ExternalInput").ap()
    if has_bfrq:
        bfrq_d = nc.dram_tensor("bfrq", [1, HW], f32r, kind="ExternalInput").ap()
    if has_bsv:
        bsv_d = nc.dram_tensor("bsv", [1, C], f32r, kind="ExternalInput").ap()
    if has_bfv:
        bfv_d = nc.dram_tensor("bfv", [1, C], f32r, kind="ExternalInput").ap()
    os_d = nc.dram_tensor("os", [BPC, C, HW], f32, kind="ExternalOutput").ap()
    of_d = nc.dram_tensor("of", [BPC, C, HW], f32, kind="ExternalOutput").ap()

    Sqrt = mybir.ActivationFunctionType.Sqrt
    Exp = mybir.ActivationFunctionType.Exp
    SUB = mybir.AluOpType.subtract
    MUL = mybir.AluOpType.mult

    with tile.TileContext(nc) as tc:
        with tc.tile_pool(name="constp", bufs=1) as constp, \
             tc.tile_pool(name="wqkp", bufs=1) as wqkp, \
             tc.tile_pool(name="data", bufs=1) as data, \
             tc.tile_pool(name="xin", bufs=2) as xin, \
             tc.tile_pool(name="wsp", bufs=8) as wsp, \
             tc.tile_pool(name="small", bufs=4) as small, \
             tc.tile_pool(name="attp", bufs=2) as attp, \
             tc.tile_pool(name="resp", bufs=2) as resp:

            # ---- constants / weights (resident) ----
            # DMA queue split: SP(sync) = inputs + ws streams; ACT(scalar) =
            # wqk + output stores; Pool(gpsimd SWDGE) = small constants.
            wcdc_t = constp.tile([128, 4, C], f32r, name="wcdc_t")
            nc.gpsimd.dma_start(out=wcdc_t,
                                in_=wcdc_d.rearrange("(kc p) c -> p kc c", p=128))
            wsv_t = constp.tile([128, CC, C], f32r, name="wsv_t")
            nc.gpsimd.dma_start(out=wsv_t,
                                in_=wsv_d.rearrange("(kc p) c -> p kc c", p=128))
            wfv_t = constp.tile([128, CC, C], f32r, name="wfv_t")
            nc.gpsimd.dma_start(out=wfv_t,
                                in_=wfv_d.rearrange("(kc p) c -> p kc c", p=128))
            ident = constp.tile([128, 128], f32, name="ident")
            make_identity(nc, ident)
            eps_t = constp.tile([128, 1], f32, name="eps_t")
            nc.vector.memset(eps_t, EPS)
            ones_t = None
            if any_bias:
                ones_f = constp.tile([1, 128], f32, name="ones_f")
                nc.vector.memset(ones_f, 1.0)
                ones_t = constp.tile([1, 128], f32r, name="ones_t")
                nc.scalar.copy(out=ones_t, in_=ones_f)

            def _bias_tile(dram, n, nm):
                t = constp.tile([1, n], f32r, name=nm)
                nc.gpsimd.dma_start(out=t, in_=dram)
                return t

            qkb_t = _bias_tile(qkb_d, J2, "qkb_t") if has_qkb else None
            bspa_t = _bias_tile(bspa_d, HW, "bspa_t") if has_bspa else None
            bfrq_t = _bias_tile(bfrq_d, HW, "bfrq_t") if has_bfrq else None
            bsv_t = _bias_tile(bsv_d, C, "bsv_t") if has_bsv else None
            bfv_t = _bias_tile(bfv_d, C, "bfv_t") if has_bfv else None

            # wqk on the ACT HWDGE queue, split per K-chunk so stage D can
            # start before the full 8MB lands.
            wqk_t = wqkp.tile([128, NCH, J2], f32r, name="wqk_t")
            for kc in range(NCH):
                nc.scalar.dma_start(
                    out=wqk_t[:, kc, :],
                    in_=wqk_d[ds(kc * 128, 128), :])

            def _samples_body():
              for b in range(BPC):
                xs_t = xin.tile([128, CC, HW], f32r, tag="xs", name=f"xs{b}")
                nc.sync.dma_start(
                    out=xs_t, in_=xs_d[b].rearrange("(cc p) n -> p cc n", p=128))
                xf_t = xin.tile([128, CC, HW], f32r, tag="xf", name=f"xf{b}",
                                bufs=1)
                nc.sync.dma_start(
                    out=xf_t, in_=xf_d[b].rearrange("(cc p) n -> p cc n", p=128))

                # ---- A: value projections, transposed: vT = x.T @ wv.T ----
                vts = data.tile([128, NCH, C], f32r, tag="vts", name=f"vts{b}")
                vtf = data.tile([128, NCH, C], f32r, tag="vtf", name=f"vtf{b}")
                with tc.tile_pool(name="psA", bufs=2, space="PSUM") as psA:
                    for src, wv, dst, bt in ((xs_t, wsv_t, vts, bsv_t),
                                             (xf_t, wfv_t, vtf, bfv_t)):
                        for mc in range(NCH):
                            ps = psA.tile([128, C], f32, tag="ps", name="psa")
                            for kc in range(CC):
                                nc.tensor.matmul(
                                    ps, src[:, kc, ts(mc, 128)], wv[:, kc, :],
                                    start=(kc == 0),
                                    stop=(kc == CC - 1 and bt is None))
                            if bt is not None:
                                nc.tensor.matmul(ps, ones_t, bt,
                                                 start=False, stop=True)
                            nc.vector.tensor_copy(out=dst[:, mc, :], in_=ps)

                # ---- B: x = w_cdc @ [xs; xf] ----
                x_sb = data.tile([128, CC, HW], f32, tag="xc", name=f"x_sb{b}")
                with tc.tile_pool(name="psB", bufs=3, space="PSUM") as psB:
                    for cc in range(CC):
                        for nn in range(2):
                            ps = psB.tile([128, 512], f32, tag="ps", name="psb")
                            for kc in range(4):
                                src = xs_t if kc < 2 else xf_t
                                nc.tensor.matmul(
                                    ps, wcdc_t[:, kc, ts(cc, 128)],
                                    src[:, kc % 2, ds(nn * 512, 512)],
                                    start=(kc == 0), stop=(kc == 3))
                            nc.scalar.copy(out=x_sb[:, cc, ds(nn * 512, 512)],
                                           in_=ps)

                # ---- LayerNorm rows of x (in place) ----
                for cc in range(CC):
                    xr = x_sb[:, cc, :].rearrange("p (s f) -> p s f", s=2)
                    stats = small.tile([128, 2, 6], f32, tag="st", name="stats")
                    for s in range(2):
                        nc.vector.bn_stats(out=stats[:, s, :], in_=xr[:, s, :])
                    mv = small.tile([128, 2], f32, tag="mv", name="mv")
                    nc.vector.bn_aggr(out=mv, in_=stats)
                    rstd = small.tile([128, 1], f32, tag="rstd", name="rstd")
                    nc.scalar.activation(out=rstd, in_=mv[:, 1:2], func=Sqrt,
                                         bias=eps_t, scale=1.0)
                    nc.vector.reciprocal(out=rstd, in_=rstd)
                    nc.vector.tensor_scalar(
                        out=x_sb[:, cc, :], in0=x_sb[:, cc, :],
                        scalar1=mv[:, 0:1], scalar2=rstd, op0=SUB, op1=MUL)

                # ---- C: xnT = xn.T ----  (shares a slot with kT: sequential)
                xnT = data.tile([128, NCH, C], f32r, tag="tp", name=f"xnT{b}")
                with tc.tile_pool(name="psT", bufs=2, space="PSUM") as psT:
                    for cc in range(CC):
                        for dc in range(NCH):
                            pt = psT.tile([128, 128], f32, tag="pt", name="pt")
                            nc.tensor.transpose(
                                pt, x_sb[:, cc, ds(dc * 128, 128)], ident)
                            nc.scalar.copy(out=xnT[:, dc, ts(cc, 128)], in_=pt)

                # ---- D: qk = xn @ wqkTg ----
                q_t = data.tile([128, CC, HW], f32r, tag="q", name=f"q{b}")
                k_sb = data.tile([128, CC, HW], f32, tag="xc", name=f"k_sb{b}")
                with tc.tile_pool(name="psD", bufs=3, space="PSUM") as psD:
                    for cc in range(CC):
                        for nn in range(4):
                            ps = psD.tile([128, 512], f32, tag="ps", name="psd")
                            for dc in range(NCH):
                                nc.tensor.matmul(
                                    ps, xnT[:, dc, ts(cc, 128)],
                                    wqk_t[:, dc, ds(nn * 512, 512)],
                                    start=(dc == 0),
                                    stop=(dc == NCH - 1 and not has_qkb))
                            if has_qkb:
                                nc.tensor.matmul(
                                    ps, ones_t, qkb_t[:, ds(nn * 512, 512)],
                                    start=False, stop=True)
                            if nn < 2:
                                nc.scalar.copy(
                                    out=q_t[:, cc, ds(nn * 512, 512)], in_=ps)
                            else:
                                nc.vector.tensor_copy(
                                    out=k_sb[:, cc, ds((nn - 2) * 512, 512)],
                                    in_=ps)

                # ---- kT = k.T ----
                kT = data.tile([128, NCH, C], f32r, tag="tp", name=f"kT{b}")
                with tc.tile_pool(name="psT2", bufs=2, space="PSUM") as psT2:
                    for cc in range(CC):
                        for mc in range(NCH):
                            pt = psT2.tile([128, 128], f32, tag="pt", name="pt2")
                            nc.tensor.transpose(
                                pt, k_sb[:, cc, ds(mc * 128, 128)], ident)
                            nc.scalar.copy(out=kT[:, mc, ts(cc, 128)], in_=pt)

                # ---- branches ----
                for br, (wsd, lb_t, vt, out_d, x_res) in enumerate((
                        (wspa_d, bspa_t, vts, os_d, xs_t),
                        (wfrq_d, bfrq_t, vtf, of_d, xf_t))):
                    # E: kw = k @ (scale * w.T); ws chunk DMAs issued up
                    # front (bufs=4 pool) so they prefetch during earlier
                    # stages on the SP queue.
                    kw = data.tile([128, CC, HW], f32r, tag="kw",
                                   name=f"kw{b}_{br}")
                    ws_tiles = []
                    for mc in range(NCH):
                        wst = wsp.tile([128, HW], f32r, tag="ws",
                                       name=f"ws{b}_{br}_{mc}")
                        nc.sync.dma_start(out=wst,
                                          in_=wsd[ds(mc * 128, 128), :])
                        ws_tiles.append(wst)
                    with tc.tile_pool(name="psE", bufs=1, space="PSUM") as psE:
                        pse = psE.tile([128, CC, HW], f32, tag="pse", name="pse")
                        for mc in range(NCH):
                            for cc in range(CC):
                                for jj in range(2):
                                    nc.tensor.matmul(
                                        pse[:, cc, ds(jj * 512, 512)],
                                        kT[:, mc, ts(cc, 128)],
                                        ws_tiles[mc][:, ds(jj * 512, 512)],
                                        start=(mc == 0), stop=(mc == NCH - 1))
                        for cc in range(CC):
                            nc.vector.tensor_copy(out=kw[:, cc, :],
                                                  in_=pse[:, cc, :])

                    # F/G: logits -> exp(+rowsum) -> out accumulation.
                    # 1/rowsum folds into the small vT chunk, not the big att.
                    with tc.tile_pool(name="psG", bufs=1, space="PSUM") as psG, \
                         tc.tile_pool(name="psF", bufs=2, space="PSUM") as psF:
                        psg = psG.tile([128, CC, HW], f32, tag="psg", name="psg")
                        for nk in range(NCH):
                            pl = psF.tile([128, HW], f32, tag="pl", name="pl")
                            for jj in range(2):
                                for cc in range(CC):
                                    nc.tensor.matmul(
                                        pl[:, ds(jj * 512, 512)],
                                        q_t[:, cc, ts(nk, 128)],
                                        kw[:, cc, ds(jj * 512, 512)],
                                        start=(cc == 0),
                                        stop=(cc == CC - 1 and lb_t is None))
                                if lb_t is not None:
                                    nc.tensor.matmul(
                                        pl[:, ds(jj * 512, 512)], ones_t,
                                        lb_t[:, ds(jj * 512, 512)],
                                        start=False, stop=True)
                            et = attp.tile([128, HW], f32r, tag="att",
                                           name=f"et{b}_{br}_{nk}")
                            rsum = small.tile([128, 1], f32, tag="rs", name="rsum")
                            nc.scalar.activation(out=et, in_=pl, func=Exp,
                                                 accum_out=rsum)
                            rrec = small.tile([128, 1], f32, tag="rr", name="rrec")
                            nc.vector.reciprocal(out=rrec, in_=rsum)
                            vtn = small.tile([128, C], f32r, tag="vtn",
                                             name="vtn")
                            nc.vector.tensor_scalar_mul(out=vtn,
                                                        in0=vt[:, nk, :],
                                                        scalar1=rrec)
                            for cc in range(CC):
                                for jj in range(2):
                                    nc.tensor.matmul(
                                        psg[:, cc, ds(jj * 512, 512)],
                                        vtn[:, ts(cc, 128)],
                                        et[:, ds(jj * 512, 512)],
                                        start=(nk == 0), stop=(nk == NCH - 1))
                        for cc in range(CC):
                            res = resp.tile([128, HW], f32, tag="res",
                                            name=f"res{b}_{br}_{cc}", bufs=1)
                            nc.vector.tensor_add(out=res, in0=psg[:, cc, :],
                                                 in1=x_res[:, cc, :].bitcast(f32))
                            nc.scalar.dma_start(
                                out=out_d[b, ds(cc * 128, 128), :], in_=res)

            if reps == 1:
                _samples_body()
            elif isinstance(reps, tuple):      # ("unroll", R)
                for _rep in range(reps[1]):
                    _samples_body()
            else:
                with tc.For_i(0, reps, 1):
                    _samples_body()

    nc.compile()
    return nc


def kernel(x_spa, x_freq, w_cdc, b_cdc, w_sv, b_sv, w_fv, b_fv,
           ln_w, ln_b, w_qk, w_spa, b_spa, w_frq, b_frq):
    x_spa = np.asarray(x_spa, np.float32)
    x_freq = np.asarray(x_freq, np.float32)
    w_cdc = np.asarray(w_cdc, np.float32)
    w_sv = np.asarray(w_sv, np.float32)
    w_fv = np.asarray(w_fv, np.float32)
    ln_w = np.asarray(ln_w, np.float32)
    ln_b = np.asarray(ln_b, np.float32)
    w_qk = np.asarray(w_qk, np.float32)
    w_spa = np.asarray(w_spa, np.float32)
    w_frq = np.asarray(w_frq, np.float32)
    b_sv = np.asarray(b_sv, np.float32)
    b_fv = np.asarray(b_fv, np.float32)
    b_spa = np.asarray(b_spa, np.float32)
    b_frq = np.asarray(b_frq, np.float32)
    # b_cdc is a per-row constant added before LayerNorm over that row: no-op.

    scale = float(HW) ** -0.5
    qkb = ln_b @ w_qk.T                      # [2hw]
    flags = (bool(np.any(qkb)), bool(np.any(b_spa)), bool(np.any(b_frq)),
             bool(np.any(b_sv)), bool(np.any(b_fv)))

    if flags not in _CACHE:
        _CACHE[flags] = _build(flags)
    nc = _CACHE[flags]

    xs = _round_f32r(x_spa.reshape(B, C, HW))
    xf = _round_f32r(x_freq.reshape(B, C, HW))
    base = {
        "wcdcT": _round_f32r(w_cdc.T),
        "wsvT": _round_f32r(w_sv.T),
        "wfvT": _round_f32r(w_fv.T),
        "wqkTg": _round_f32r(w_qk.T * ln_w[:, None]),
        "wspaT": _round_f32r(w_spa.T * scale),
        "wfrqT": _round_f32r(w_frq.T * scale),
    }
    if flags[0]:
        base["qkb"] = _round_f32r(qkb[None, :])
    if flags[1]:
        base["bspa"] = _round_f32r(b_spa[None, :])
    if flags[2]:
        base["bfrq"] = _round_f32r(b_frq[None, :])
    if flags[3]:
        base["bsv"] = _round_f32r(b_sv[None, :])
    if flags[4]:
        base["bfv"] = _round_f32r(b_fv[None, :])

    in_maps = []
    for c in range(NCORES):
        m = dict(base)
        m["xs"] = xs[c * BPC:(c + 1) * BPC]
        m["xf"] = xf[c * BPC:(c + 1) * BPC]
        in_maps.append(m)

    res = bass_utils.run_bass_kernel_spmd(nc, in_maps, core_ids=list(range(NCORES)))
    out_spa = np.concatenate([res.results[c]["os"] for c in range(NCORES)], axis=0)
    out_frq = np.concatenate([res.results[c]["of"] for c in range(NCORES)], axis=0)
    return (out_spa.reshape(B, C, H, W).astype(np.float32),
            out_frq.reshape(B, C, H, W).astype(np.float32))


# revision 20
# speedup vs baseline: 135.1346x; 135.1346x over previous
"""Trainium2 Bass kernel for nn_CMIA_2843268350555 (dual-branch spatial/freq attention).

Strategy: data-parallel over batch (16 samples / 8 cores = 2 per core).
All matmuls in float32r (11-bit mantissa, full PE rate at free-dim>=256).

Per-sample math (C=256 channels, HW=1024):
  vT_b    = (x_b.T @ w_bv.T)            [hw, c]   (b in {spa, frq})
  x       = w_cdc @ [x_spa; x_frq]      [c, hw]   (+b_cdc: no-op through LN)
  xn      = layernorm_rows(x)           [c, hw]   (affine folded into wqkTg)
  xnT     = transpose(xn)               [hw, c]
  qk      = (xnT.T @ wqkTg) = xn@..     [c, 2hw]  -> q [c,hw], k [c,hw]
  kT      = transpose(k)                [hw, c]
  kw_b    = (kT.T @ (scale*w_b.T))      [c, hw]   (associativity: avoids big logits mm)
  logits  = q.T @ kw_b                  [hw(n), hw(j)]
  att_b   = softmax_j(logits + b_b)
  out_b   = x_b + (vT_b.T @ att_b)      [c, hw]
"""
import numpy as np
import ml_dtypes

import concourse.bacc as bacc
import concourse.mybir as mybir
import concourse.tile as tile
from concourse import bass_utils
from concourse.bass import ts, ds
from concourse.masks import make_identity

f32 = mybir.dt.float32
f32r = mybir.dt.float32r
bf16 = mybir.dt.bfloat16

# bf16 for the streamed attention-branch weights (and kT): halves the
# dominant per-sample DMA stream, but costs ~3e-3 scale-rel output error
# vs 4e-4 with f32r. Measured equal-within-noise on HW, so keep f32r.
WS_BF16 = False
WS_DT = bf16 if WS_BF16 else f32r

B, C, H, W = 16, 256, 32, 32
HW = H * W           # 1024
J2 = 2 * HW          # 2048
NCORES = 8
BPC = B // NCORES    # samples per core
CC = C // 128        # 2 channel chunks
NCH = HW // 128      # 8 hw chunks
EPS = 1e-5


def _round_f32r(x: np.ndarray) -> np.ndarray:
    """RNE-round fp32 to fp32r (11 mantissa bits; low 12 bits zero)."""
    x = np.ascontiguousarray(x, dtype=np.float32)
    u = x.view(np.uint32)
    lsb = (u >> np.uint32(12)) & np.uint32(1)
    r = u + np.uint32(0x7FF) + lsb
    return (r & ~np.uint32(0xFFF)).view(np.float32)


_CACHE: dict = {}


def _ws_prep(x: np.ndarray) -> np.ndarray:
    if WS_BF16:
        return np.ascontiguousarray(x, np.float32).astype(ml_dtypes.bfloat16)
    return _round_f32r(x)


def _build(flags, reps=1):
    has_qkb, has_bspa, has_bfrq, has_bsv, has_bfv = flags
    any_bias = any(flags)

    nc = bacc.Bacc("TRN2", target_bir_lowering=False, debug=False,
                   enable_asserts=True, num_devices=NCORES)
    xs_d = nc.dram_tensor("xs", [BPC, C, HW], f32r, kind="ExternalInput").ap()
    xf_d = nc.dram_tensor("xf", [BPC, C, HW], f32r, kind="ExternalInput").ap()
    wcdc_d = nc.dram_tensor("wcdcT", [2 * C, C], f32r, kind="ExternalInput").ap()
    wsv_d = nc.dram_tensor("wsvT", [C, C], f32r, kind="ExternalInput").ap()
    wfv_d = nc.dram_tensor("wfvT", [C, C], f32r, kind="ExternalInput").ap()
    wqk_d = nc.dram_tensor("wqkTg", [HW, J2], f32r, kind="ExternalInput").ap()
    wspa_d = nc.dram_tensor("wspaT", [HW, HW], WS_DT, kind="ExternalInput").ap()
    wfrq_d = nc.dram_tensor("wfrqT", [HW, HW], WS_DT, kind="ExternalInput").ap()
    qkb_d = bspa_d = bfrq_d = bsv_d = bfv_d = None
    if has_qkb:
        qkb_d = nc.dram_tensor("qkb", [1, J2], f32r, kind="ExternalInput").ap()
    if has_bspa:
        bspa_d = nc.dram_tensor("bspa", [1, HW], f32r, kind="ExternalInput").ap()
    if has_bfrq:
        bfrq_d = nc.dram_tensor("bfrq", [1, HW], f32r, kind="ExternalInput").ap()
    if has_bsv:
        bsv_d = nc.dram_tensor("bsv", [1, C], f32r, kind="ExternalInput").ap()
    if has_bfv:
        bfv_d = nc.dram_tensor("bfv", [1, C], f32r, kind="ExternalInput").ap()
    os_d = nc.dram_tensor("os", [BPC, C, HW], f32, kind="ExternalOutput").ap()
    of_d = nc.dram_tensor("of", [BPC, C, HW], f32, kind="ExternalOutput").ap()

    Sqrt = mybir.ActivationFunctionType.Sqrt
    Exp = mybir.ActivationFunctionType.Exp
    SUB = mybir.AluOpType.subtract
    MUL = mybir.AluOpType.mult

    with tile.TileContext(nc) as tc:
        with tc.tile_pool(name="constp", bufs=1) as constp, \
             tc.tile_pool(name="wqkp", bufs=1) as wqkp, \
             tc.tile_pool(name="data", bufs=1) as data, \
             tc.tile_pool(name="xin", bufs=2) as xin, \
             tc.tile_pool(name="wsp", bufs=8) as wsp, \
             tc.tile_pool(name="small", bufs=4) as small, \
             tc.tile_pool(name="attp", bufs=2) as attp, \
             tc.tile_pool(name="resp", bufs=2) as resp:

            # ---- constants / weights (resident) ----
            # DMA queue split: SP(sync) = inputs + ws streams; ACT(scalar) =
            # wqk + output stores; Pool(gpsimd SWDGE) = small constants.
            wcdc_t = constp.tile([128, 4, C], f32r, name="wcdc_t")
            nc.gpsimd.dma_start(out=wcdc_t,
                                in_=wcdc_d.rearrange("(kc p) c -> p kc c", p=128))
            wsv_t = constp.tile([128, CC, C], f32r, name="wsv_t")
            nc.gpsimd.dma_start(out=wsv_t,
                                in_=wsv_d.rearrange("(kc p) c -> p kc c", p=128))
            wfv_t = constp.tile([128, CC, C], f32r, name="wfv_t")
            nc.gpsimd.dma_start(out=wfv_t,
                                in_=wfv_d.rearrange("(kc p) c -> p kc c", p=128))
            ident = constp.tile([128, 128], f32, name="ident")
            make_identity(nc, ident)
            eps_t = constp.tile([128, 1], f32, name="eps_t")
            nc.vector.memset(eps_t, EPS)
            ones_t = None
            if any_bias:
                ones_f = constp.tile([1, 128], f32, name="ones_f")
                nc.vector.memset(ones_f, 1.0)
                ones_t = constp.tile([1, 128], f32r, name="ones_t")
                nc.scalar.copy(out=ones_t, in_=ones_f)

            def _bias_tile(dram, n, nm):
                t = constp.tile([1, n], f32r, name=nm)
                nc.gpsimd.dma_start(out=t, in_=dram)
                return t

            qkb_t = _bias_tile(qkb_d, J2, "qkb_t") if has_qkb else None
            bspa_t = _bias_tile(bspa_d, HW, "bspa_t") if has_bspa else None
            bfrq_t = _bias_tile(bfrq_d, HW, "bfrq_t") if has_bfrq else None
            bsv_t = _bias_tile(bsv_d, C, "bsv_t") if has_bsv else None
            bfv_t = _bias_tile(bfv_d, C, "bfv_t") if has_bfv else None

            # wqk split across the ACT HWDGE queue and the Pool SWDGE queue
            # (SP stays free for inputs/ws) so stage D's K-chunks land early.
            wqk_t = wqkp.tile([128, NCH, J2], f32r, name="wqk_t")
            for kc in range(NCH):
                eng = nc.scalar if kc < 4 else nc.gpsimd
                eng.dma_start(
                    out=wqk_t[:, kc, :],
                    in_=wqk_d[ds(kc * 128, 128), :])

            def _samples_body():
              for b in range(BPC):
                xs_t = xin.tile([128, CC, HW], f32r, tag="xs", name=f"xs{b}")
                nc.sync.dma_start(
                    out=xs_t, in_=xs_d[b].rearrange("(cc p) n -> p cc n", p=128))
                xf_t = xin.tile([128, CC, HW], f32r, tag="xf", name=f"xf{b}",
                                bufs=1)
                nc.sync.dma_start(
                    out=xf_t, in_=xf_d[b].rearrange("(cc p) n -> p cc n", p=128))

                vts = data.tile([128, NCH, C], f32r, tag="vts", name=f"vts{b}")
                vtf = data.tile([128, NCH, C], f32r, tag="vtf", name=f"vtf{b}")
                x_sb = data.tile([128, CC, HW], f32, tag="xc", name=f"x_sb{b}")
                xnT = data.tile([128, NCH, C], f32r, tag="tp", name=f"xnT{b}")

                # One shared matmul-psum pool (3x512) + transpose pool (2)
                # across stages A-D avoids per-stage PSUM zone churn.
                with tc.tile_pool(name="psMM", bufs=3, space="PSUM") as psMM, \
                     tc.tile_pool(name="psT", bufs=2, space="PSUM") as psT:
                    # ---- A: value projections, transposed ----
                    for src, wv, dst, bt in ((xs_t, wsv_t, vts, bsv_t),
                                             (xf_t, wfv_t, vtf, bfv_t)):
                        for mc in range(NCH):
                            ps = psMM.tile([128, 512], f32, tag="ps", name="psa")
                            for kc in range(CC):
                                nc.tensor.matmul(
                                    ps[:, 0:C],
                                    src[:, kc, ts(mc, 128)], wv[:, kc, :],
                                    start=(kc == 0),
                                    stop=(kc == CC - 1 and bt is None))
                            if bt is not None:
                                nc.tensor.matmul(ps[:, 0:C], ones_t, bt,
                                                 start=False, stop=True)
                            nc.vector.tensor_copy(out=dst[:, mc, :],
                                                  in_=ps[:, 0:C])

                    # ---- B: x = w_cdc @ [xs; xf] ----
                    for cc in range(CC):
                        for nn in range(2):
                            ps = psMM.tile([128, 512], f32, tag="ps", name="psb")
                            for kc in range(4):
                                src = xs_t if kc < 2 else xf_t
                                nc.tensor.matmul(
                                    ps, wcdc_t[:, kc, ts(cc, 128)],
                                    src[:, kc % 2, ds(nn * 512, 512)],
                                    start=(kc == 0), stop=(kc == 3))
                            nc.scalar.copy(out=x_sb[:, cc, ds(nn * 512, 512)],
                                           in_=ps)

                    # ---- LayerNorm rows of x (in place) ----
                    for cc in range(CC):
                        xr = x_sb[:, cc, :].rearrange("p (s f) -> p s f", s=2)
                        stats = small.tile([128, 2, 6], f32, tag="st",
                                           name="stats")
                        for s in range(2):
                            nc.vector.bn_stats(out=stats[:, s, :],
                                               in_=xr[:, s, :])
                        mv = small.tile([128, 2], f32, tag="mv", name="mv")
                        nc.vector.bn_aggr(out=mv, in_=stats)
                        rstd = small.tile([128, 1], f32, tag="rstd", name="rstd")
                        nc.scalar.activation(out=rstd, in_=mv[:, 1:2], func=Sqrt,
                                             bias=eps_t, scale=1.0)
                        nc.vector.reciprocal(out=rstd, in_=rstd)
                        nc.vector.tensor_scalar(
                            out=x_sb[:, cc, :], in0=x_sb[:, cc, :],
                            scalar1=mv[:, 0:1], scalar2=rstd, op0=SUB, op1=MUL)

                    # ---- C: xnT = xn.T ----  (xnT shares slot with kT)
                    for cc in range(CC):
                        for dc in range(NCH):
                            pt = psT.tile([128, 128], f32, tag="pt", name="pt")
                            nc.tensor.transpose(
                                pt, x_sb[:, cc, ds(dc * 128, 128)], ident)
                            nc.scalar.copy(out=xnT[:, dc, ts(cc, 128)], in_=pt)

                    # ---- D: qk = xn @ wqkTg ----
                    q_t = data.tile([128, CC, HW], f32r, tag="q", name=f"q{b}")
                    k_sb = data.tile([128, CC, HW], f32, tag="xc",
                                     name=f"k_sb{b}")
                    for cc in range(CC):
                        for nn in range(4):
                            ps = psMM.tile([128, 512], f32, tag="ps", name="psd")
                            for dc in range(NCH):
                                nc.tensor.matmul(
                                    ps, xnT[:, dc, ts(cc, 128)],
                                    wqk_t[:, dc, ds(nn * 512, 512)],
                                    start=(dc == 0),
                                    stop=(dc == NCH - 1 and not has_qkb))
                            if has_qkb:
                                nc.tensor.matmul(
                                    ps, ones_t, qkb_t[:, ds(nn * 512, 512)],
                                    start=False, stop=True)
                            if nn < 2:
                                nc.scalar.copy(
                                    out=q_t[:, cc, ds(nn * 512, 512)], in_=ps)
                            else:
                                nc.vector.tensor_copy(
                                    out=k_sb[:, cc, ds((nn - 2) * 512, 512)],
                                    in_=ps)

                    # ---- kT = k.T ----
                    kT = data.tile([128, NCH, C], WS_DT, tag="tp", name=f"kT{b}")
                    for cc in range(CC):
                        for mc in range(NCH):
                            pt = psT.tile([128, 128], f32, tag="pt", name="pt2")
                            nc.tensor.transpose(
                                pt, k_sb[:, cc, ds(mc * 128, 128)], ident)
                            nc.scalar.copy(out=kT[:, mc, ts(cc, 128)], in_=pt)

                # ---- branches ----
                for br, (wsd, lb_t, vt, out_d, x_res) in enumerate((
                        (wspa_d, bspa_t, vts, os_d, xs_t),
                        (wfrq_d, bfrq_t, vtf, of_d, xf_t))):
                    # E: kw = k @ (scale * w.T); ws chunk DMAs issued up
                    # front (bufs=4 pool) so they prefetch during earlier
                    # stages on the SP queue.
                    kw = data.tile([128, CC, HW], f32r, tag="kw",
                                   name=f"kw{b}_{br}")
                    ws_tiles = []
                    for mc in range(NCH):
                        wst = wsp.tile([128, HW], WS_DT, tag="ws",
                                       name=f"ws{b}_{br}_{mc}")
                        nc.sync.dma_start(out=wst,
                                          in_=wsd[ds(mc * 128, 128), :])
                        ws_tiles.append(wst)
                    with tc.tile_pool(name="psE", bufs=1, space="PSUM") as psE:
                        pse = psE.tile([128, CC, HW], f32, tag="pse", name="pse")
                        for mc in range(NCH):
                            for cc in range(CC):
                                for jj in range(2):
                                    nc.tensor.matmul(
                                        pse[:, cc, ds(jj * 512, 512)],
                                        kT[:, mc, ts(cc, 128)],
                                        ws_tiles[mc][:, ds(jj * 512, 512)],
                                        start=(mc == 0), stop=(mc == NCH - 1))
                        for cc in range(CC):
                            for jj in range(2):
                                cp = (nc.vector.tensor_copy if jj == 0
                                      else nc.scalar.copy)
                                cp(out=kw[:, cc, ds(jj * 512, 512)],
                                   in_=pse[:, cc, ds(jj * 512, 512)])

                    # F/G: logits -> exp(+rowsum) -> out accumulation.
                    # 1/rowsum folds into the small vT chunk, not the big att.
                    # psF opened first => lower PSUM zone => next branch's psE
                    # reuses the logits banks (freed early), not psg's.
                    with tc.tile_pool(name="psF", bufs=2, space="PSUM") as psF, \
                         tc.tile_pool(name="psG", bufs=1, space="PSUM") as psG:
                        psg = psG.tile([128, CC, HW], f32, tag="psg", name="psg")
                        for nk in range(NCH):
                            pl = psF.tile([128, HW], f32, tag="pl", name="pl")
                            for jj in range(2):
                                for cc in range(CC):
                                    nc.tensor.matmul(
                                        pl[:, ds(jj * 512, 512)],
                                        q_t[:, cc, ts(nk, 128)],
                                        kw[:, cc, ds(jj * 512, 512)],
                                        start=(cc == 0),
                                        stop=(cc == CC - 1 and lb_t is None))
                                if lb_t is not None:
                                    nc.tensor.matmul(
                                        pl[:, ds(jj * 512, 512)], ones_t,
                                        lb_t[:, ds(jj * 512, 512)],
                                        start=False, stop=True)
                            et = attp.tile([128, HW], f32r, tag="att",
                                           name=f"et{b}_{br}_{nk}")
                            rsum = small.tile([128, 1], f32, tag="rs", name="rsum")
                            nc.scalar.activation(out=et, in_=pl, func=Exp,
                                                 accum_out=rsum)
                            rrec = small.tile([128, 1], f32, tag="rr", name="rrec")
                            nc.vector.reciprocal(out=rrec, in_=rsum)
                            vtn = small.tile([128, C], f32r, tag="vtn",
                                             name="vtn")
                            nc.vector.tensor_scalar_mul(out=vtn,
                                                        in0=vt[:, nk, :],
                                                        scalar1=rrec)
                            for cc in range(CC):
                                for jj in range(2):
                                    nc.tensor.matmul(
                                        psg[:, cc, ds(jj * 512, 512)],
                                        vtn[:, ts(cc, 128)],
                                        et[:, ds(jj * 512, 512)],
                                        start=(nk == 0), stop=(nk == NCH - 1))
                        for cc in range(CC):
                            res = resp.tile([128, HW], f32, tag="res",
                                            name=f"res{b}_{br}_{cc}", bufs=1)
                            nc.vector.tensor_add(out=res, in0=psg[:, cc, :],
                                                 in1=x_res[:, cc, :].bitcast(f32))
                            nc.scalar.dma_start(
                                out=out_d[b, ds(cc * 128, 128), :], in_=res)

            if reps == 1:
                _samples_body()
            elif isinstance(reps, tuple):      # ("unroll", R)
                for _rep in range(reps[1]):
                    _samples_body()
            else:
                with tc.For_i(0, reps, 1):
                    _samples_body()

    nc.compile()
    return nc


def kernel(x_spa, x_freq, w_cdc, b_cdc, w_sv, b_sv, w_fv, b_fv,
           ln_w, ln_b, w_qk, w_spa, b_spa, w_frq, b_frq):
    x_spa = np.asarray(x_spa, np.float32)
    x_freq = np.asarray(x_freq, np.float32)
    w_cdc = np.asarray(w_cdc, np.float32)
    w_sv = np.asarray(w_sv, np.float32)
    w_fv = np.asarray(w_fv, np.float32)
    ln_w = np.asarray(ln_w, np.float32)
    ln_b = np.asarray(ln_b, np.float32)
    w_qk = np.asarray(w_qk, np.float32)
    w_spa = np.asarray(w_spa, np.float32)
    w_frq = np.asarray(w_frq, np.float32)
    b_sv = np.asarray(b_sv, np.float32)
    b_fv = np.asarray(b_fv, np.float32)
    b_spa = np.asarray(b_spa, np.float32)
    b_frq = np.asarray(b_frq, np.float32)
    # b_cdc is a per-row constant added before LayerNorm over that row: no-op.

    scale = float(HW) ** -0.5
    qkb = ln_b @ w_qk.T                      # [2hw]
    flags = (bool(np.any(qkb)), bool(np.any(b_spa)), bool(np.any(b_frq)),
             bool(np.any(b_sv)), bool(np.any(b_fv)))

    if flags not in _CACHE:
        _CACHE[flags] = _build(flags)
    nc = _CACHE[flags]

    xs = _round_f32r(x_spa.reshape(B, C, HW))
    xf = _round_f32r(x_freq.reshape(B, C, HW))
    base = {
        "wcdcT": _round_f32r(w_cdc.T),
        "wsvT": _round_f32r(w_sv.T),
        "wfvT": _round_f32r(w_fv.T),
        "wqkTg": _round_f32r(w_qk.T * ln_w[:, None]),
        "wspaT": _ws_prep(w_spa.T * scale),
        "wfrqT": _ws_prep(w_frq.T * scale),
    }
    if flags[0]:
        base["qkb"] = _round_f32r(qkb[None, :])
    if flags[1]:
        base["bspa"] = _round_f32r(b_spa[None, :])
    if flags[2]:
        base["bfrq"] = _round_f32r(b_frq[None, :])
    if flags[3]:
        base["bsv"] = _round_f32r(b_sv[None, :])
    if flags[4]:
        base["bfv"] = _round_f32r(b_fv[None, :])

    in_maps = []
    for c in range(NCORES):
        m = dict(base)
        m["xs"] = xs[c * BPC:(c + 1) * BPC]
        m["xf"] = xf[c * BPC:(c + 1) * BPC]
        in_maps.append(m)

    res = bass_utils.run_bass_kernel_spmd(nc, in_maps, core_ids=list(range(NCORES)))
    out_spa = np.concatenate([res.results[c]["os"] for c in range(NCORES)], axis=0)
    out_frq = np.concatenate([res.results[c]["of"] for c in range(NCORES)], axis=0)
    return (out_spa.reshape(B, C, H, W).astype(np.float32),
            out_frq.reshape(B, C, H, W).astype(np.float32))
